# revision 12
# baseline (speedup 1.0000x reference)
"""CMHSA kernel for 8x TRN2 NeuronCores.

Sharding: data-parallel over the batch dim (B=8 -> one batch per core), no
collectives. Each core runs the full attention pipeline for its batch:

  xf = x[b] as [C, T]
  q = (scale*Wq) @ xf, k = Wk @ xf           (lhsT = host-pretransposed W)
  vT = xf^T @ Wv^T  -> [T, C]                (lhsT = xf, moving = Wv^T)
  per output head g:
    ST[t, q] = sum_{h,d} k[(h,d), t] * (head_w[g,h] * q[(h,d), q])
               (head conv fused into the score matmul via a K=384 stacked
               contraction; per-partition scaling of q on DVE)
    E = exp(ST), Esq = exp(2*ST)             (ACT, straight from PSUM)
    AV matmul lhsT = [vT_g | ones]: rows 0..63 = E^T@v_g, row 64 = softmax
    denominators; ones-matmul on Esq -> per-q sum of E^2
  instance-norm folded into an affine on the AV output:
    mean == 1/T exactly (softmax rows sum to 1)
    var from sum(E^2)/denom^2; rsqrt via reciprocal+sqrt + one Newton step
    out_g^T = (OT0 * (1/denom)) * rN + cN * colsum(v_g)
  colsum(v) comes from rowsum(x) pushed through the V projection.
  projection consumes the torch-style .view(B,T,C) reshape via stride-6
  access patterns over UT = concat_g out_g^T  (no data movement)
  yT[c_out, t] = sum_cb projW_shuf[:, cb] @ UT[:, cb::6] + projb

float32r notes (probed against walrus): matmul inputs must be produced as
f32r (DMA from f32r DRAM, or DVE/ACT ops with f32r out); f32r matmuls
require dst base partition 0 and an even moving free dim.
"""

import numpy as np
from contextlib import ExitStack

import concourse.bass as bass
import concourse.bacc as bacc
import concourse.tile as tile
from concourse import mybir
from concourse.bass_utils import run_bass_kernel_spmd

B, C, H, W = 8, 384, 32, 32
NH, HD = 6, 64
T = H * W              # 1024
P = 128                # partitions
NB = C // P            # 3 channel blocks
TBN = T // P           # 8 t-blocks
EPS = 1e-5
SCALE = HD ** -0.5

F32 = mybir.dt.float32
F32R = mybir.dt.float32r
AF = mybir.ActivationFunctionType
OP = mybir.AluOpType
AX = mybir.AxisListType

# matmul input dtype: float32r streams 1 row/cycle (vs 4 for float32)
MM_DT = F32


def build_kernel(tc, debug=False):
    nc = tc.nc
    ctx = ExitStack()

    xf_d = nc.dram_tensor("xf", [C, T], MM_DT, kind="ExternalInput").ap()
    wqt_d = nc.dram_tensor("wqt", [C, C], MM_DT, kind="ExternalInput").ap()
    wkt_d = nc.dram_tensor("wkt", [C, C], MM_DT, kind="ExternalInput").ap()
    wvt_d = nc.dram_tensor("wvt", [C, C], MM_DT, kind="ExternalInput").ap()
    pwts_d = nc.dram_tensor("pwts", [HD, NH * C], MM_DT, kind="ExternalInput").ap()
    vto_d = nc.dram_tensor("vto", [P, NH + 1], MM_DT, kind="ExternalInput").ap()
    wvec_d = nc.dram_tensor("wvec", [P, NH * NB], F32, kind="ExternalInput").ap()
    gam_d = nc.dram_tensor("gam", [NH, 1], F32, kind="ExternalInput").ap()
    bet_d = nc.dram_tensor("bet", [NH, 1], F32, kind="ExternalInput").ap()
    pjb_d = nc.dram_tensor("pjb", [P, NB], F32, kind="ExternalInput").ap()
    yt_d = nc.dram_tensor("yt", [C, T], F32, kind="ExternalOutput").ap()
    if debug:
        dbg = {
            "d_q0": nc.dram_tensor("d_q0", [P, T], F32, kind="ExternalOutput").ap(),
            "d_k0": nc.dram_tensor("d_k0", [P, T], F32, kind="ExternalOutput").ap(),
            "d_vta0": nc.dram_tensor("d_vta0", [P, NH * (HD + 1)], F32, kind="ExternalOutput").ap(),
            "d_statsd": nc.dram_tensor("d_statsd", [NH, 2 * T], F32, kind="ExternalOutput").ap(),
            "d_rd": nc.dram_tensor("d_rd", [NH, T], F32, kind="ExternalOutput").ap(),
            "d_vsum": nc.dram_tensor("d_vsum", [HD, NH], F32, kind="ExternalOutput").ap(),
            "d_rncn": nc.dram_tensor("d_rncn", [1, 2 * NH], F32, kind="ExternalOutput").ap(),
            "d_ut": nc.dram_tensor("d_ut", [HD, NH * T], F32, kind="ExternalOutput").ap(),
            "d_utraw": nc.dram_tensor("d_utraw", [HD, NH * T], F32, kind="ExternalOutput").ap(),
            "d_et50": nc.dram_tensor("d_et50", [P, T], F32, kind="ExternalOutput").ap(),
        }

    cons = ctx.enter_context(tc.tile_pool(name="cons", bufs=1))
    sb = ctx.enter_context(tc.tile_pool(name="sb", bufs=1))
    work = ctx.enter_context(tc.tile_pool(name="work", bufs=1))
    pp = ctx.enter_context(tc.tile_pool(name="pp", bufs=1, space="PSUM"))

    # ---- constant / persistent tiles -------------------------------------
    wqt = [cons.tile([P, C], MM_DT, tag=f"wqt{i}", name=f"wqt{i}") for i in range(NB)]
    wkt = [cons.tile([P, C], MM_DT, tag=f"wkt{i}", name=f"wkt{i}") for i in range(NB)]
    wvt = [cons.tile([P, C], MM_DT, tag=f"wvt{i}", name=f"wvt{i}") for i in range(NB)]
    pwts = cons.tile([HD, NH * C], MM_DT, tag="pwts")
    onesr = cons.tile([P, 2], MM_DT, tag="onesr")
    wvec = cons.tile([P, NH * NB], F32, tag="wvec")
    gam = cons.tile([NH, 1], F32, tag="gam")
    bet = cons.tile([NH, 1], F32, tag="bet")
    pjb = cons.tile([P, NB], F32, tag="pjb")

    xf = [sb.tile([P, T], MM_DT, tag=f"xf{i}", name=f"xf{i}") for i in range(NB)]
    qsb = [sb.tile([P, T], F32, tag=f"q{i}", name=f"q{i}") for i in range(NB)]
    ksb = [sb.tile([P, T], MM_DT, tag=f"k{i}", name=f"k{i}") for i in range(NB)]
    # vta[tb]: per head g, cols [g*65, g*65+64) = vT slice, col g*65+64 = 1.0
    vta = [sb.tile([P, NH * (HD + 1)], MM_DT, tag=f"vta{i}", name=f"vta{i}")
           for i in range(TBN)]
    ut = sb.tile([HD, NH * T], MM_DT, tag="ut")
    statsd = sb.tile([NH, 2 * T], F32, tag="statsd")   # [:, :T] denom, [:, T:] sumsq
    rd = sb.tile([NH, T], F32, tag="rd")
    vsum_sb = sb.tile([HD, NH], F32, tag="vsum_sb")
    rncn_row = sb.tile([1, 2 * NH], F32, tag="rncn_row")

    for i in range(NB):
        nc.sync.dma_start(xf[i][:, :], xf_d[i * P:(i + 1) * P, :])
        nc.sync.dma_start(wqt[i][:, :], wqt_d[i * P:(i + 1) * P, :])
        nc.sync.dma_start(wkt[i][:, :], wkt_d[i * P:(i + 1) * P, :])
        nc.sync.dma_start(wvt[i][:, :], wvt_d[i * P:(i + 1) * P, :])
    nc.sync.dma_start(pwts[:, :], pwts_d[:, :])
    nc.sync.dma_start(onesr[:, :], vto_d[:, 0:2])
    nc.sync.dma_start(wvec[:, :], wvec_d[:, :])
    nc.sync.dma_start(gam[:, :], gam_d[:, :])
    nc.sync.dma_start(bet[:, :], bet_d[:, :])
    nc.sync.dma_start(pjb[:, :], pjb_d[:, :])

    # ---- stage 1: Q/K projections [C, T]; V^T projection [T, C] ----------
    for mb in range(NB):
        for dst, wt in ((qsb, wqt), (ksb, wkt)):
            ps = pp.tile([P, T], F32, tag="st", bufs=2)
            for qh in range(2):
                for kb in range(NB):
                    nc.tensor.matmul(
                        ps[:, qh * 512:(qh + 1) * 512],
                        lhsT=wt[kb][:, mb * P:(mb + 1) * P],
                        rhs=xf[kb][:, qh * 512:(qh + 1) * 512],
                        start=(kb == 0), stop=(kb == NB - 1),
                    )
            nc.vector.tensor_copy(dst[mb][:, :], ps[:, :])

    for tb in range(TBN):
        ps = pp.tile([P, C], F32, tag="st", bufs=2)
        for kb in range(NB):
            nc.tensor.matmul(
                ps[:, :],
                lhsT=xf[kb][:, tb * P:(tb + 1) * P],
                rhs=wvt[kb][:, :],
                start=(kb == 0), stop=(kb == NB - 1),
            )
        # scatter v columns into the [vT_g | 1] interleaved layout
        vdst = vta[tb].rearrange("p (g c) -> p g c", c=HD + 1)
        nc.vector.tensor_copy(vdst[:, :, 0:HD], ps[:, :])
        nc.sync.dma_start(vdst[:, :, HD], vto_d[:, 0:NH])

    if debug:
        nc.sync.dma_start(dbg["d_q0"][:, :], qsb[0][:, :])
        nc.sync.dma_start(dbg["d_k0"][:, :], ksb[0].bitcast(F32)[:, :])
        nc.sync.dma_start(dbg["d_vta0"][:, :], vta[0].bitcast(F32)[:, :])

    # colsum(v)[c] = sum_c' xsum[c'] * WvT[c', c], xsum = rowsum(x)
    vs_ps = pp.tile([1, C], F32, tag="st", bufs=2, name="vs_ps")
    for kb in range(NB):
        xs32 = work.tile([P, 1], F32, tag="xs32", name="xs32")
        nc.vector.reduce_sum(xs32[:, :], xf[kb].bitcast(F32)[:, :], axis=AX.X)
        xs = work.tile([P, 1], MM_DT, tag="xs", name="xs")
        nc.vector.tensor_copy(xs[:, :], xs32[:, :])
        nc.tensor.matmul(vs_ps[:, :], lhsT=xs[:, :], rhs=wvt[kb][:, :],
                         start=(kb == 0), stop=(kb == NB - 1))
    vsrow = work.tile([1, C], F32, tag="vsrow")
    nc.vector.tensor_copy(vsrow[:, :], vs_ps[:, :])
    # [1, 384] row -> [64, 6] (partition=d, free=g): SBUF->SBUF partition
    # scatter is illegal, so bounce through DRAM where APs are unrestricted
    vsd = nc.dram_tensor("vsd", [1, C], F32, kind="Internal").ap()
    nc.sync.dma_start(vsd[:, :], vsrow[:, :])
    vsr = vsd.rearrange("p (g d) -> p g d", d=HD)
    nc.sync.dma_start(vsum_sb[:, :], vsr[0, :, :].transpose([1, 0]))

    # ---- stage 2: per output head: scores + softmax + AV -----------------
    for g in range(NH):
        qq = [work.tile([P, T], MM_DT, tag=f"qq{kb}", bufs=2, name=f"qq{kb}")
              for kb in range(NB)]
        for kb in range(NB):
            nc.vector.tensor_scalar(
                qq[kb][:, :], qsb[kb][:, :],
                scalar1=wvec[:, g * NB + kb:g * NB + kb + 1], scalar2=None,
                op0=OP.mult,
            )
        av = pp.tile([HD + 1, T], F32, tag="av", bufs=1)
        psq = pp.tile([1, T], F32, tag="sq", bufs=1)
        for tb in range(TBN):
            st = pp.tile([P, T], F32, tag="st", bufs=2)
            for qh in range(2):
                for kb in range(NB):
                    nc.tensor.matmul(
                        st[:, qh * 512:(qh + 1) * 512],
                        lhsT=ksb[kb][:, tb * P:(tb + 1) * P],
                        rhs=qq[kb][:, qh * 512:(qh + 1) * 512],
                        start=(kb == 0), stop=(kb == NB - 1),
                    )
            et = work.tile([P, T], MM_DT, tag="et", bufs=3)
            esq = work.tile([P, T], MM_DT, tag="esq", bufs=3)
            nc.scalar.activation(et[:, :], st[:, :], AF.Exp)
            nc.scalar.activation(esq[:, :], st[:, :], AF.Exp, scale=2.0)
            if debug and g == 5 and tb == 0:
                nc.sync.dma_start(dbg["d_et50"][:, :], et.bitcast(F32)[:, :])
            for qh in range(2):
                sl = slice(qh * 512, (qh + 1) * 512)
                nc.tensor.matmul(
                    av[0:HD + 1, sl],
                    lhsT=vta[tb][:, g * (HD + 1):(g + 1) * (HD + 1)],
                    rhs=et[:, sl],
                    start=(tb == 0), stop=(tb == TBN - 1),
                    skip_group_check=True,
                )
                nc.tensor.matmul(
                    psq[0:1, sl],
                    lhsT=onesr[:, 0:1],
                    rhs=esq[:, sl],
                    start=(tb == 0), stop=(tb == TBN - 1),
                    skip_group_check=True,
                )
        # stats rows -> SBUF staging -> partition g of the batched tile
        stg = work.tile([HD + 1, T], F32, tag="stg", bufs=2)
        nc.vector.tensor_copy(stg[HD:HD + 1, :], av[HD:HD + 1, :])
        nc.vector.tensor_copy(stg[0:1, :], psq[0:1, :])
        nc.sync.dma_start(statsd[g:g + 1, 0:T], stg[HD:HD + 1, :])
        nc.sync.dma_start(statsd[g:g + 1, T:2 * T], stg[0:1, :])
        nc.vector.tensor_copy(ut[:, g * T:(g + 1) * T], av[0:HD, :])

    if debug:
        nc.sync.dma_start(dbg["d_utraw"][:, :], ut.bitcast(F32)[:, :])
        nc.sync.dma_start(dbg["d_statsd"][:, :], statsd[:, :])

    # ---- stage 3: batched instance-norm stats ----------------------------
    nc.vector.reciprocal(rd[:, :], statsd[:, 0:T])
    t6a = work.tile([NH, T], F32, tag="t6a")
    nc.vector.tensor_tensor(t6a[:, :], rd[:, :], rd[:, :], op=OP.mult)
    nc.vector.tensor_tensor(t6a[:, :], t6a[:, :], statsd[:, T:2 * T], op=OP.mult)
    s2 = work.tile([NH, 1], F32, tag="s2")
    nc.vector.reduce_sum(s2[:, :], t6a[:, :], axis=AX.X)
    var_e = work.tile([NH, 1], F32, tag="var_e")  # var + EPS
    nc.vector.tensor_scalar(
        var_e[:, :], s2[:, :],
        scalar1=1.0 / (T * T), scalar2=(EPS - 1.0 / (T * T)),
        op0=OP.mult, op1=OP.add,
    )
    inv_ve = work.tile([NH, 1], F32, tag="inv_ve")
    nc.vector.reciprocal(inv_ve[:, :], var_e[:, :])
    r0 = work.tile([NH, 1], F32, tag="r0")
    nc.scalar.activation(r0[:, :], inv_ve[:, :], AF.Sqrt)
    # one Newton step: r1 = r0 * (1.5 - 0.5 * ve * r0^2)
    t1 = work.tile([NH, 1], F32, tag="t1")
    nc.vector.tensor_tensor(t1[:, :], r0[:, :], r0[:, :], op=OP.mult)
    nc.vector.tensor_tensor(t1[:, :], t1[:, :], var_e[:, :], op=OP.mult)
    nc.vector.tensor_scalar(t1[:, :], t1[:, :], scalar1=-0.5, scalar2=1.5,
                            op0=OP.mult, op1=OP.add)
    rn = work.tile([NH, 2], F32, tag="rn")
    nc.vector.tensor_tensor(t1[:, :], t1[:, :], r0[:, :], op=OP.mult)
    nc.vector.tensor_tensor(rn[:, 0:1], t1[:, :], gam[:, :], op=OP.mult)
    # cN = beta - rN/T
    nc.vector.tensor_scalar(rn[:, 1:2], rn[:, 0:1], scalar1=-1.0 / T, scalar2=None,
                            op0=OP.mult)
    nc.vector.tensor_tensor(rn[:, 1:2], rn[:, 1:2], bet[:, :], op=OP.add)
    nc.sync.dma_start(rncn_row[:, :], rn[:, :])

    if debug:
        nc.sync.dma_start(dbg["d_rd"][:, :], rd[:, :])
        nc.sync.dma_start(dbg["d_vsum"][:, :], vsum_sb[:, :])
        nc.sync.dma_start(dbg["d_rncn"][:, :], rncn_row[:, :])

    # ---- stage 4: apply normalization affine to UT -----------------------
    for g in range(NH):
        rdg = work.tile([1, T], F32, tag="rdg", bufs=2)
        nc.sync.dma_start(rdg[:, :], rd[g:g + 1, :])
        rdbc = work.tile([HD, T], F32, tag="rdbc", bufs=2)
        nc.gpsimd.partition_broadcast(rdbc[:, :], rdg[:, :])
        rnbc = work.tile([HD, 1], F32, tag="rnbc", bufs=2)
        cnbc = work.tile([HD, 1], F32, tag="cnbc", bufs=2)
        nc.gpsimd.partition_broadcast(rnbc[:, :], rncn_row[:, 2 * g:2 * g + 1])
        nc.gpsimd.partition_broadcast(cnbc[:, :], rncn_row[:, 2 * g + 1:2 * g + 2])
        avec = work.tile([HD, 1], F32, tag="avec", bufs=2)
        nc.vector.tensor_tensor(avec[:, :], vsum_sb[:, g:g + 1], cnbc[:, :],
                                op=OP.mult)
        usl = ut[:, g * T:(g + 1) * T]
        nc.vector.tensor_tensor(usl, usl.bitcast(F32), rdbc[:, :], op=OP.mult)
        nc.vector.tensor_scalar(usl, usl.bitcast(F32), scalar1=rnbc[:, :],
                                scalar2=avec[:, :], op0=OP.mult, op1=OP.add)

    if debug:
        nc.sync.dma_start(dbg["d_ut"][:, :], ut.bitcast(F32)[:, :])

    # ---- stage 5: output projection via stride-6 APs ---------------------
    utr = ut.rearrange("p (t s) -> p t s", s=NH)
    for mb in range(NB):
        yps = pp.tile([P, T], F32, tag="av", bufs=1, name="yps")
        for qh in range(2):
            for cb in range(NH):
                nc.tensor.matmul(
                    yps[:, qh * 512:(qh + 1) * 512],
                    lhsT=pwts[:, cb * C + mb * P:cb * C + (mb + 1) * P],
                    rhs=utr[:, qh * 512:(qh + 1) * 512, cb],
                    start=(cb == 0), stop=(cb == NH - 1),
                )
        ysb = work.tile([P, T], F32, tag="ysb", bufs=2)
        nc.vector.tensor_scalar(ysb[:, :], yps[:, :],
                                scalar1=pjb[:, mb:mb + 1], scalar2=None, op0=OP.add)
        nc.sync.dma_start(yt_d[mb * P:(mb + 1) * P, :], ysb[:, :])

    ctx.close()


_CACHED = {}


def _get_nc(debug=False):
    if debug not in _CACHED:
        nc = bacc.Bacc("TRN2", target_bir_lowering=False, debug=False,
                       num_devices=B)
        with tile.TileContext(nc) as tc:
            build_kernel(tc, debug=debug)
        nc.compile()
        _CACHED[debug] = nc
    return _CACHED[debug]


def prep_inputs(x, Wq, Wk, Wv, head_w, gamma, beta, projW, projb):
    x = np.ascontiguousarray(x, dtype=np.float32)
    xfs = x.reshape(B, C, T)
    wqt = np.ascontiguousarray((Wq * SCALE).T, dtype=np.float32)
    wkt = np.ascontiguousarray(Wk.T, dtype=np.float32)
    wvt = np.ascontiguousarray(Wv.T, dtype=np.float32)
    pwts = np.empty((HD, NH * C), dtype=np.float32)
    for cb in range(NH):
        pwts[:, cb * C:(cb + 1) * C] = projW[:, cb * HD:(cb + 1) * HD].T
    vto = np.ones((P, NH + 1), dtype=np.float32)
    wvec = np.empty((P, NH * NB), dtype=np.float32)
    for g in range(NH):
        for kb in range(NB):
            rows = (kb * P + np.arange(P)) // HD
            wvec[:, g * NB + kb] = head_w[g, rows]
    gam = np.ascontiguousarray(gamma.reshape(NH, 1), dtype=np.float32)
    bet = np.ascontiguousarray(beta.reshape(NH, 1), dtype=np.float32)
    pjb = np.ascontiguousarray(projb.reshape(NB, P).T, dtype=np.float32)
    shared = dict(wqt=wqt, wkt=wkt, wvt=wvt, pwts=pwts, vto=vto, wvec=wvec,
                  gam=gam, bet=bet, pjb=pjb)
    return [dict(xf=np.ascontiguousarray(xfs[i]), **shared) for i in range(B)]


def run(in_maps, debug=False, **kw):
    nc = _get_nc(debug=debug)
    return run_bass_kernel_spmd(nc, in_maps, core_ids=list(range(B)), **kw)


def kernel(**inputs):
    in_maps = prep_inputs(**inputs)
    res = run(in_maps)
    out = np.stack([res.results[i]["yt"].reshape(C, H, W) for i in range(B)])
    return out.astype(np.float32)
